# revision 11
# baseline (speedup 1.0000x reference)
"""Trainium2 kernel for nn_Communication_64467459113042.

Reference computation:
    out[c,h,w] = (1/N) * sum_n cpf[c,n] * g[n,h,w]
    g[n,h,w]   = ((w - cx_n)^2 + (h - cy_n)^2) / (2 * bev_n^2)

The gaussian-exponent map is a rank-4 polynomial in (h, w), so the einsum
collapses exactly to

    out[c,h,w] = A[c]*(h^2 + w^2) + B[c] + Cw[c]*w + Ch[c]*h

with per-channel coefficients that are O(C*N) reductions:
    s_n   = 1 / (2 * bev_n^2)
    A[c]  =  (1/N) sum_n cpf[c,n] * s_n
    B[c]  =  (1/N) sum_n cpf[c,n] * s_n * (cx_n^2 + cy_n^2)
    Cw[c] = -(2/N) sum_n cpf[c,n] * s_n * cx_n
    Ch[c] = -(2/N) sum_n cpf[c,n] * s_n * cy_n

The tiny coefficient/gather stage (~100 KB of reads) runs on host; the
device kernel evaluates the [C=256, HW=65536] rank-4 matmul

    out = Mmat.T @ basis,  Mmat [4, 256], basis [4, HW]

sharded over H across the 8 cores (each core owns h in [32*i, 32*i+32) and
writes a disjoint [256, 8192] slice — no collective needed).
"""

import numpy as np

N_CORES = 8
C = 256
H = 256
W = 256
MAX_N = 100
H_SHARD = H // N_CORES          # 32 rows of the image per core
COLS = H_SHARD * W              # 8192 flat (h,w) columns per core
FREE = 512                      # matmul moving free dim (one PSUM bank)
CHUNK = 2048                    # output DMA chunk, [128, 2048] f32 = 1 MiB

_CACHE = {}


K_ROWS = 10                     # 5 exact fp16 basis rows x (hi, lo) coeff split


N_CHUNKS = (C // 128) * (COLS // CHUNK)   # 16 output chunks of [128, CHUNK]


def _get_nc():
    """Raw bacc program (no TileContext: avoids its entry/exit barrier and
    EVSEM fabric, worth ~15-20 us on a ~25 us kernel).

    Pipeline, chunk k = (cblk, jo):
      PE:   4 matmuls [K_ROWS,128]x[K_ROWS,512] -> psum group k%2 (4 banks)
      DVE (even k) / ACT (odd k): copy psum group -> sbuf slot k%4
      Sync: DMA sbuf slot -> out chunk (1 MiB each)
    """
    if "nc" in _CACHE:
        return _CACHE["nc"]

    from contextlib import ExitStack

    import concourse.mybir as mybir
    from concourse import bacc

    f32 = mybir.dt.float32
    f16 = mybir.dt.float16
    nc = bacc.Bacc("TRN2", target_bir_lowering=False, debug=False,
                   num_devices=N_CORES)
    mm = nc.dram_tensor("mmat", [K_ROWS, C], f16, kind="ExternalInput").ap()
    bs = nc.dram_tensor("basis", [K_ROWS, COLS], f16, kind="ExternalInput").ap()
    out = nc.dram_tensor("out", [C, COLS], f32, kind="ExternalOutput").ap()

    HALF = CHUNK // 2
    n_jo = COLS // CHUNK

    with ExitStack() as ctx:
        wt = ctx.enter_context(nc.sbuf_tensor("wt", [K_ROWS, C], f16))
        bt = ctx.enter_context(nc.sbuf_tensor("bt", [K_ROWS, COLS], f16))
        ob = [ctx.enter_context(nc.sbuf_tensor(f"ob{i}", [128, CHUNK], f32))
              for i in range(N_CHUNKS)]
        ps = [ctx.enter_context(nc.psum_tensor(f"ps{i}", [128, CHUNK], f32))
              for i in range(2)]
        dma_w = ctx.enter_context(nc.semaphore("dma_w"))
        dma_b = ctx.enter_context(nc.semaphore("dma_b"))
        mmsem = ctx.enter_context(nc.semaphore("mmsem"))
        vcp = ctx.enter_context(nc.semaphore("vcp"))
        scp = ctx.enter_context(nc.semaphore("scp"))
        dout = ctx.enter_context(nc.semaphore("dout"))
        block = ctx.enter_context(nc.Block())

        # weights on the gpsimd ring, basis quarters on the sync ring:
        # both issue concurrently right after the preamble, and chunk k's
        # matmuls only gate on basis quarter k%4 (sync-ring FIFO order
        # makes dma_b >= 16*(q+1) imply quarters 0..q have landed)
        @block.gpsimd
        def _(gpsimd):
            gpsimd.dma_start(wt[:], mm[:]).then_inc(dma_w, 16)

        @block.sync
        def _(sync):
            for q in range(n_jo):
                sync.dma_start(bt[:, q * CHUNK:(q + 1) * CHUNK],
                               bs[:, q * CHUNK:(q + 1) * CHUNK]).then_inc(dma_b, 16)
            # chunk 0 goes out as two half-DMAs so the write wire starts as
            # soon as the first half-copy lands; later chunks are full-size
            sync.wait_ge(vcp, 1)
            sync.dma_start(out[0:128, 0:HALF], ob[0][:, :HALF]).then_inc(dout, 16)
            sync.wait_ge(scp, 1)
            sync.dma_start(out[0:128, HALF:CHUNK], ob[0][:, HALF:]).then_inc(dout, 16)
            for k in range(1, N_CHUNKS):
                cblk, jo = divmod(k, n_jo)
                sync.wait_ge(vcp, k + 1)
                sync.wait_ge(scp, k + 1)
                sync.dma_start(
                    out[cblk * 128:(cblk + 1) * 128,
                        jo * CHUNK:(jo + 1) * CHUNK],
                    ob[k][:],
                ).then_inc(dout, 16)
            sync.wait_ge(dout, 16 * (N_CHUNKS + 1))

        # mmsem +1 per matmul PAIR (2 per chunk) so each half-copy starts
        # as soon as its two banks are written
        @block.tensor
        def _(tensor):
            tensor.wait_ge(dma_w, 16)
            for k in range(N_CHUNKS):
                cblk, jo = divmod(k, n_jo)
                g = k % 2
                if k < n_jo:
                    tensor.wait_ge(dma_b, 16 * (jo + 1))
                if k >= 2:
                    # psum group g freed once chunk k-2 fully copied
                    tensor.wait_ge(vcp, k - 1)
                    tensor.wait_ge(scp, k - 1)
                for ji in range(CHUNK // FREE):
                    col = jo * CHUNK + ji * FREE
                    ins = nc.tensor.matmul(
                        ps[g][:, ji * FREE:(ji + 1) * FREE],
                        wt[:, cblk * 128:(cblk + 1) * 128],
                        bt[:, col:col + FREE],
                        start=True, stop=True,
                    )
                    if ji % 2 == 1:
                        ins.then_inc(mmsem, 1)

        # each chunk drains via two concurrent half-copies: DVE takes the
        # low banks, ACT the high banks
        @block.vector
        def _(vector):
            for k in range(N_CHUNKS):
                vector.wait_ge(mmsem, 2 * k + 1)
                nc.vector.tensor_copy(
                    ob[k][:, :HALF], ps[k % 2][:, :HALF]).then_inc(vcp, 1)

        @block.scalar
        def _(scalar):
            for k in range(N_CHUNKS):
                scalar.wait_ge(mmsem, 2 * k + 2)
                nc.scalar.copy(
                    ob[k][:, HALF:], ps[k % 2][:, HALF:]).then_inc(scp, 1)

    nc.compile()
    _CACHE["nc"] = nc
    return nc


def _host_coeffs(pred_box_infra, pred_score_infra, infra_features):
    """Replicates the reference's box->(cx, cy, bev, cpf) pipeline in f32,
    then reduces to the rank-4 coefficient matrix Mmat [4, C] in f64."""
    pred_box_infra = np.asarray(pred_box_infra, dtype=np.float32)
    pred_score_infra = np.asarray(pred_score_infra, dtype=np.float32)
    feat = np.asarray(infra_features, dtype=np.float32)[0]  # [C,H,W]

    idx = np.nonzero(pred_score_infra > 0.0)[0][:MAX_N]
    if len(idx) < MAX_N:
        idx = np.concatenate([idx, np.zeros(MAX_N - len(idx), dtype=idx.dtype)])

    boxes = pred_box_infra[idx]          # [N,8,3]
    l = boxes.min(axis=1)
    r = boxes.max(axis=1)
    center = (l + r) / 2.0

    lx = (l[:, 0] + W / 2.0) / 160.0
    ly = (l[:, 1] + H / 2.0) / 160.0
    rx = (r[:, 0] + W / 2.0) / 160.0
    ry = (r[:, 1] + H / 2.0) / 160.0
    bev = (ry - ly) * (rx - lx)
    cx = (center[:, 0] + W / 2.0) / 160.0
    cy = (center[:, 1] + H / 2.0) / 160.0

    # bilinear grid-sample of feat at (cx, cy), f32 to match the reference
    ix = ((cx + 1.0) * W - 1.0) / 2.0
    iy = ((cy + 1.0) * H - 1.0) / 2.0
    x0 = np.floor(ix)
    y0 = np.floor(iy)
    x1 = x0 + 1.0
    y1 = y0 + 1.0
    wx1 = ix - x0
    wx0 = np.float32(1.0) - wx1
    wy1 = iy - y0
    wy0 = np.float32(1.0) - wy1

    def gather(xf, yf):
        xi = xf.astype(np.int32)
        yi = yf.astype(np.int32)
        valid = (xi >= 0) & (xi < W) & (yi >= 0) & (yi < H)
        xi = np.clip(xi, 0, W - 1)
        yi = np.clip(yi, 0, H - 1)
        return feat[:, yi, xi] * valid.astype(feat.dtype)

    cpf = (gather(x0, y0) * (wx0 * wy0) + gather(x1, y0) * (wx1 * wy0)
           + gather(x0, y1) * (wx0 * wy1) + gather(x1, y1) * (wx1 * wy1))

    cx64 = cx.astype(np.float64)
    cy64 = cy.astype(np.float64)
    s = 1.0 / (2.0 * bev.astype(np.float64) ** 2)
    cpf64 = cpf.astype(np.float64)
    A = cpf64 @ s / MAX_N
    B = cpf64 @ (s * (cx64 ** 2 + cy64 ** 2)) / MAX_N
    Cw = cpf64 @ (s * cx64) * (-2.0 / MAX_N)
    Ch = cpf64 @ (s * cy64) * (-2.0 / MAX_N)
    # coefficient order pairs the basis rows [r2/4 (fp16 hi), r2_lo, 1, w, h]
    coefs = np.stack([4.0 * A, A, B, Cw, Ch])          # [5, C] f64
    hi = coefs.astype(np.float16)
    lo = (coefs - hi.astype(np.float64)).astype(np.float16)
    return np.ascontiguousarray(np.concatenate([hi, lo], axis=0))  # [10, C]


def _bases():
    """Per-core fp16 basis [K_ROWS, COLS] over the core's h-slice.

    Rows (repeated for the hi and lo coefficient halves):
      q   = fp16(r2/4)  -- exactly representable (multiple of 16, <= 32640)
      r2l = r2 - 4*q    -- integer, |r2l| <= 32, exact in fp16
      1, w, h           -- integers <= 255, exact in fp16
    so the only quantization error in out = Mmat.T @ basis is the fp16
    hi/lo split of the coefficients (~2^-22 relative)."""
    if "bases" in _CACHE:
        return _CACHE["bases"]
    w = np.arange(W, dtype=np.float64)
    h = np.arange(H, dtype=np.float64)
    r2 = h[:, None] ** 2 + w[None, :] ** 2            # [H, W] integers
    q = (r2 / 4.0).astype(np.float16)
    r2l = r2 - 4.0 * q.astype(np.float64)
    basis = np.empty((5, H, W), dtype=np.float16)
    basis[0] = q
    basis[1] = r2l
    basis[2] = 1.0
    basis[3] = w[None, :]
    basis[4] = h[:, None]
    basis = np.concatenate([basis, basis], axis=0)    # [K_ROWS, H, W]
    bases = [
        np.ascontiguousarray(
            basis[:, i * H_SHARD:(i + 1) * H_SHARD, :].reshape(K_ROWS, COLS))
        for i in range(N_CORES)
    ]
    _CACHE["bases"] = bases
    return bases


def _device_run(mmat, trace=False):
    from concourse.bass_utils import run_bass_kernel_spmd

    nc = _get_nc()
    bases = _bases()
    in_maps = [{"mmat": mmat, "basis": bases[i]} for i in range(N_CORES)]
    return run_bass_kernel_spmd(nc, in_maps, core_ids=list(range(N_CORES)),
                                trace=trace)


def kernel(pred_box_infra, pred_score_infra, infra_features):
    mmat = _host_coeffs(pred_box_infra, pred_score_infra, infra_features)
    res = _device_run(mmat)
    parts = [res.results[i]["out"].reshape(C, H_SHARD, W)
             for i in range(N_CORES)]
    return np.concatenate(parts, axis=1).reshape(1, C, H, W)


# revision 12
# speedup vs baseline: 1.1131x; 1.1131x over previous
"""Trainium2 kernel for nn_Communication_64467459113042.

Reference computation:
    out[c,h,w] = (1/N) * sum_n cpf[c,n] * g[n,h,w]
    g[n,h,w]   = ((w - cx_n)^2 + (h - cy_n)^2) / (2 * bev_n^2)

The gaussian-exponent map is a rank-4 polynomial in (h, w), so the einsum
collapses exactly to

    out[c,h,w] = A[c]*(h^2 + w^2) + B[c] + Cw[c]*w + Ch[c]*h

with per-channel coefficients that are O(C*N) reductions:
    s_n   = 1 / (2 * bev_n^2)
    A[c]  =  (1/N) sum_n cpf[c,n] * s_n
    B[c]  =  (1/N) sum_n cpf[c,n] * s_n * (cx_n^2 + cy_n^2)
    Cw[c] = -(2/N) sum_n cpf[c,n] * s_n * cx_n
    Ch[c] = -(2/N) sum_n cpf[c,n] * s_n * cy_n

The tiny coefficient/gather stage (~100 KB of reads) runs on host; the
device kernel evaluates the [C=256, HW=65536] rank-4 matmul

    out = Mmat.T @ basis,  Mmat [4, 256], basis [4, HW]

sharded over H across the 8 cores (each core owns h in [32*i, 32*i+32) and
writes a disjoint [256, 8192] slice — no collective needed).
"""

import numpy as np

N_CORES = 8
C = 256
H = 256
W = 256
MAX_N = 100
H_SHARD = H // N_CORES          # 32 rows of the image per core
COLS = H_SHARD * W              # 8192 flat (h,w) columns per core
FREE = 512                      # matmul moving free dim (one PSUM bank)
CHUNK = 2048                    # output DMA chunk, [128, 2048] f32 = 1 MiB

_CACHE = {}


K_ROWS = 10                     # 5 exact fp16 basis rows x (hi, lo) coeff split


N_CHUNKS = (C // 128) * (COLS // CHUNK)   # 16 output chunks of [128, CHUNK]


def _get_nc():
    """Raw bacc program (no TileContext: avoids its entry/exit barrier and
    EVSEM fabric, worth ~15-20 us on a ~25 us kernel).

    Pipeline, chunk k = (cblk, jo):
      PE:   4 matmuls [K_ROWS,128]x[K_ROWS,512] -> psum group k%2 (4 banks)
      DVE (even k) / ACT (odd k): copy psum group -> sbuf slot k%4
      Sync: DMA sbuf slot -> out chunk (1 MiB each)
    """
    if "nc" in _CACHE:
        return _CACHE["nc"]

    from contextlib import ExitStack

    import concourse.bass as bass_mod
    import concourse.mybir as mybir
    from concourse import bacc

    f32 = mybir.dt.float32
    f16 = mybir.dt.float16
    # Skip the post-preamble all-engine barrier Bass.__init__ emits after its
    # const-AP memsets: our engine streams are fully semaphore-guarded and
    # nothing reads the const APs until ~5 us after the memsets retire, so
    # the barrier only delays the first input DMA by ~2.5 us. The Block-exit
    # barrier (emitted later, outside this patch) is kept.
    _orig_barrier = bass_mod.Bass.all_engine_barrier
    bass_mod.Bass.all_engine_barrier = lambda self, **kw: None
    try:
        nc = bacc.Bacc("TRN2", target_bir_lowering=False, debug=False,
                       num_devices=N_CORES)
    finally:
        bass_mod.Bass.all_engine_barrier = _orig_barrier
    mm = nc.dram_tensor("mmat", [K_ROWS, C], f16, kind="ExternalInput").ap()
    bs = nc.dram_tensor("basis", [K_ROWS, COLS], f16, kind="ExternalInput").ap()
    out = nc.dram_tensor("out", [C, COLS], f32, kind="ExternalOutput").ap()

    HALF = CHUNK // 2
    n_jo = COLS // CHUNK

    with ExitStack() as ctx:
        wt = ctx.enter_context(nc.sbuf_tensor("wt", [K_ROWS, C], f16))
        bt = ctx.enter_context(nc.sbuf_tensor("bt", [K_ROWS, COLS], f16))
        ob = [ctx.enter_context(nc.sbuf_tensor(f"ob{i}", [128, CHUNK], f32))
              for i in range(N_CHUNKS)]
        ps = [ctx.enter_context(nc.psum_tensor(f"ps{i}", [128, CHUNK], f32))
              for i in range(2)]
        dma_w = ctx.enter_context(nc.semaphore("dma_w"))
        dma_b = ctx.enter_context(nc.semaphore("dma_b"))
        mmsem = ctx.enter_context(nc.semaphore("mmsem"))
        vcp = ctx.enter_context(nc.semaphore("vcp"))
        scp = ctx.enter_context(nc.semaphore("scp"))
        dout = ctx.enter_context(nc.semaphore("dout"))
        block = ctx.enter_context(nc.Block())

        # weights on the gpsimd ring, basis quarters on the sync ring:
        # both issue concurrently right after the preamble, and chunk k's
        # matmuls only gate on basis quarter k%4 (sync-ring FIFO order
        # makes dma_b >= 16*(q+1) imply quarters 0..q have landed)
        @block.gpsimd
        def _(gpsimd):
            gpsimd.dma_start(wt[:], mm[:]).then_inc(dma_w, 16)

        @block.sync
        def _(sync):
            for q in range(n_jo):
                sync.dma_start(bt[:, q * CHUNK:(q + 1) * CHUNK],
                               bs[:, q * CHUNK:(q + 1) * CHUNK]).then_inc(dma_b, 16)
            # chunk 0 goes out as two half-DMAs so the write wire starts as
            # soon as the first half-copy lands; later chunks are full-size
            sync.wait_ge(vcp, 1)
            sync.dma_start(out[0:128, 0:HALF], ob[0][:, :HALF]).then_inc(dout, 16)
            sync.wait_ge(scp, 1)
            sync.dma_start(out[0:128, HALF:CHUNK], ob[0][:, HALF:]).then_inc(dout, 16)
            for k in range(1, N_CHUNKS):
                cblk, jo = divmod(k, n_jo)
                sync.wait_ge(vcp, k + 1)
                sync.wait_ge(scp, k + 1)
                sync.dma_start(
                    out[cblk * 128:(cblk + 1) * 128,
                        jo * CHUNK:(jo + 1) * CHUNK],
                    ob[k][:],
                ).then_inc(dout, 16)
            sync.wait_ge(dout, 16 * (N_CHUNKS + 1))

        # mmsem +1 per matmul PAIR (2 per chunk) so each half-copy starts
        # as soon as its two banks are written
        @block.tensor
        def _(tensor):
            tensor.wait_ge(dma_w, 16)
            for k in range(N_CHUNKS):
                cblk, jo = divmod(k, n_jo)
                g = k % 2
                if k < n_jo:
                    tensor.wait_ge(dma_b, 16 * (jo + 1))
                if k >= 2:
                    # psum group g freed once chunk k-2 fully copied
                    tensor.wait_ge(vcp, k - 1)
                    tensor.wait_ge(scp, k - 1)
                for ji in range(CHUNK // FREE):
                    col = jo * CHUNK + ji * FREE
                    ins = nc.tensor.matmul(
                        ps[g][:, ji * FREE:(ji + 1) * FREE],
                        wt[:, cblk * 128:(cblk + 1) * 128],
                        bt[:, col:col + FREE],
                        start=True, stop=True,
                    )
                    if ji % 2 == 1:
                        ins.then_inc(mmsem, 1)

        # each chunk drains via two concurrent half-copies: DVE takes the
        # low banks, ACT the high banks
        @block.vector
        def _(vector):
            for k in range(N_CHUNKS):
                vector.wait_ge(mmsem, 2 * k + 1)
                nc.vector.tensor_copy(
                    ob[k][:, :HALF], ps[k % 2][:, :HALF]).then_inc(vcp, 1)

        @block.scalar
        def _(scalar):
            for k in range(N_CHUNKS):
                scalar.wait_ge(mmsem, 2 * k + 2)
                nc.scalar.copy(
                    ob[k][:, HALF:], ps[k % 2][:, HALF:]).then_inc(scp, 1)

    nc.compile()
    _CACHE["nc"] = nc
    return nc


def _host_coeffs(pred_box_infra, pred_score_infra, infra_features):
    """Replicates the reference's box->(cx, cy, bev, cpf) pipeline in f32,
    then reduces to the rank-4 coefficient matrix Mmat [4, C] in f64."""
    pred_box_infra = np.asarray(pred_box_infra, dtype=np.float32)
    pred_score_infra = np.asarray(pred_score_infra, dtype=np.float32)
    feat = np.asarray(infra_features, dtype=np.float32)[0]  # [C,H,W]

    idx = np.nonzero(pred_score_infra > 0.0)[0][:MAX_N]
    if len(idx) < MAX_N:
        idx = np.concatenate([idx, np.zeros(MAX_N - len(idx), dtype=idx.dtype)])

    boxes = pred_box_infra[idx]          # [N,8,3]
    l = boxes.min(axis=1)
    r = boxes.max(axis=1)
    center = (l + r) / 2.0

    lx = (l[:, 0] + W / 2.0) / 160.0
    ly = (l[:, 1] + H / 2.0) / 160.0
    rx = (r[:, 0] + W / 2.0) / 160.0
    ry = (r[:, 1] + H / 2.0) / 160.0
    bev = (ry - ly) * (rx - lx)
    cx = (center[:, 0] + W / 2.0) / 160.0
    cy = (center[:, 1] + H / 2.0) / 160.0

    # bilinear grid-sample of feat at (cx, cy), f32 to match the reference
    ix = ((cx + 1.0) * W - 1.0) / 2.0
    iy = ((cy + 1.0) * H - 1.0) / 2.0
    x0 = np.floor(ix)
    y0 = np.floor(iy)
    x1 = x0 + 1.0
    y1 = y0 + 1.0
    wx1 = ix - x0
    wx0 = np.float32(1.0) - wx1
    wy1 = iy - y0
    wy0 = np.float32(1.0) - wy1

    def gather(xf, yf):
        xi = xf.astype(np.int32)
        yi = yf.astype(np.int32)
        valid = (xi >= 0) & (xi < W) & (yi >= 0) & (yi < H)
        xi = np.clip(xi, 0, W - 1)
        yi = np.clip(yi, 0, H - 1)
        return feat[:, yi, xi] * valid.astype(feat.dtype)

    cpf = (gather(x0, y0) * (wx0 * wy0) + gather(x1, y0) * (wx1 * wy0)
           + gather(x0, y1) * (wx0 * wy1) + gather(x1, y1) * (wx1 * wy1))

    cx64 = cx.astype(np.float64)
    cy64 = cy.astype(np.float64)
    s = 1.0 / (2.0 * bev.astype(np.float64) ** 2)
    cpf64 = cpf.astype(np.float64)
    A = cpf64 @ s / MAX_N
    B = cpf64 @ (s * (cx64 ** 2 + cy64 ** 2)) / MAX_N
    Cw = cpf64 @ (s * cx64) * (-2.0 / MAX_N)
    Ch = cpf64 @ (s * cy64) * (-2.0 / MAX_N)
    # coefficient order pairs the basis rows [r2/4 (fp16 hi), r2_lo, 1, w, h]
    coefs = np.stack([4.0 * A, A, B, Cw, Ch])          # [5, C] f64
    hi = coefs.astype(np.float16)
    lo = (coefs - hi.astype(np.float64)).astype(np.float16)
    return np.ascontiguousarray(np.concatenate([hi, lo], axis=0))  # [10, C]


def _bases():
    """Per-core fp16 basis [K_ROWS, COLS] over the core's h-slice.

    Rows (repeated for the hi and lo coefficient halves):
      q   = fp16(r2/4)  -- exactly representable (multiple of 16, <= 32640)
      r2l = r2 - 4*q    -- integer, |r2l| <= 32, exact in fp16
      1, w, h           -- integers <= 255, exact in fp16
    so the only quantization error in out = Mmat.T @ basis is the fp16
    hi/lo split of the coefficients (~2^-22 relative)."""
    if "bases" in _CACHE:
        return _CACHE["bases"]
    w = np.arange(W, dtype=np.float64)
    h = np.arange(H, dtype=np.float64)
    r2 = h[:, None] ** 2 + w[None, :] ** 2            # [H, W] integers
    q = (r2 / 4.0).astype(np.float16)
    r2l = r2 - 4.0 * q.astype(np.float64)
    basis = np.empty((5, H, W), dtype=np.float16)
    basis[0] = q
    basis[1] = r2l
    basis[2] = 1.0
    basis[3] = w[None, :]
    basis[4] = h[:, None]
    basis = np.concatenate([basis, basis], axis=0)    # [K_ROWS, H, W]
    bases = [
        np.ascontiguousarray(
            basis[:, i * H_SHARD:(i + 1) * H_SHARD, :].reshape(K_ROWS, COLS))
        for i in range(N_CORES)
    ]
    _CACHE["bases"] = bases
    return bases


def _device_run(mmat, trace=False):
    from concourse.bass_utils import run_bass_kernel_spmd

    nc = _get_nc()
    bases = _bases()
    in_maps = [{"mmat": mmat, "basis": bases[i]} for i in range(N_CORES)]
    return run_bass_kernel_spmd(nc, in_maps, core_ids=list(range(N_CORES)),
                                trace=trace)


def kernel(pred_box_infra, pred_score_infra, infra_features):
    mmat = _host_coeffs(pred_box_infra, pred_score_infra, infra_features)
    res = _device_run(mmat)
    parts = [res.results[i]["out"].reshape(C, H_SHARD, W)
             for i in range(N_CORES)]
    return np.concatenate(parts, axis=1).reshape(1, C, H, W)


# revision 15
# speedup vs baseline: 1.1148x; 1.0015x over previous
"""Trainium2 kernel for nn_Communication_64467459113042.

Reference computation:
    out[c,h,w] = (1/N) * sum_n cpf[c,n] * g[n,h,w]
    g[n,h,w]   = ((w - cx_n)^2 + (h - cy_n)^2) / (2 * bev_n^2)

The gaussian-exponent map is a rank-4 polynomial in (h, w), so the einsum
collapses exactly to

    out[c,h,w] = A[c]*(h^2 + w^2) + B[c] + Cw[c]*w + Ch[c]*h

with per-channel coefficients that are O(C*N) reductions:
    s_n   = 1 / (2 * bev_n^2)
    A[c]  =  (1/N) sum_n cpf[c,n] * s_n
    B[c]  =  (1/N) sum_n cpf[c,n] * s_n * (cx_n^2 + cy_n^2)
    Cw[c] = -(2/N) sum_n cpf[c,n] * s_n * cx_n
    Ch[c] = -(2/N) sum_n cpf[c,n] * s_n * cy_n

The tiny coefficient/gather stage (~100 KB of reads) runs on host; the
device kernel evaluates the [C=256, HW=65536] rank-4 matmul

    out = Mmat.T @ basis,  Mmat [4, 256], basis [4, HW]

sharded over H across the 8 cores (each core owns h in [32*i, 32*i+32) and
writes a disjoint [256, 8192] slice — no collective needed).
"""

import numpy as np

N_CORES = 8
C = 256
H = 256
W = 256
MAX_N = 100
H_SHARD = H // N_CORES          # 32 rows of the image per core
COLS = H_SHARD * W              # 8192 flat (h,w) columns per core
FREE = 512                      # matmul moving free dim (one PSUM bank)
CHUNK = 2048                    # output DMA chunk, [128, 2048] f32 = 1 MiB

_CACHE = {}


K_ROWS = 10                     # 5 exact fp16 basis rows x (hi, lo) coeff split


N_CHUNKS = (C // 128) * (COLS // CHUNK)   # 16 output chunks of [128, CHUNK]


def _get_nc():
    """Raw bacc program (no TileContext: avoids its entry/exit barrier and
    EVSEM fabric, worth ~15-20 us on a ~25 us kernel).

    Pipeline, chunk k = (cblk, jo):
      PE:   4 matmuls [K_ROWS,128]x[K_ROWS,512] -> psum group k%2 (4 banks)
      DVE (even k) / ACT (odd k): copy psum group -> sbuf slot k%4
      Sync: DMA sbuf slot -> out chunk (1 MiB each)
    """
    if "nc" in _CACHE:
        return _CACHE["nc"]

    from contextlib import ExitStack

    import concourse.bass as bass_mod
    import concourse.mybir as mybir
    from concourse import bacc

    f32 = mybir.dt.float32
    f16 = mybir.dt.float16
    # Skip the post-preamble all-engine barrier Bass.__init__ emits after its
    # const-AP memsets: our engine streams are fully semaphore-guarded and
    # nothing reads the const APs until ~5 us after the memsets retire, so
    # the barrier only delays the first input DMA by ~2.5 us. The Block-exit
    # barrier (emitted later, outside this patch) is kept.
    _orig_barrier = bass_mod.Bass.all_engine_barrier
    bass_mod.Bass.all_engine_barrier = lambda self, **kw: None
    try:
        nc = bacc.Bacc("TRN2", target_bir_lowering=False, debug=False,
                       num_devices=N_CORES)
    finally:
        bass_mod.Bass.all_engine_barrier = _orig_barrier
    mm = nc.dram_tensor("mmat", [K_ROWS, C], f16, kind="ExternalInput").ap()
    bs = nc.dram_tensor("basis", [K_ROWS, COLS], f16, kind="ExternalInput").ap()
    out = nc.dram_tensor("out", [C, COLS], f32, kind="ExternalOutput").ap()

    HALF = CHUNK // 2
    n_jo = COLS // CHUNK

    with ExitStack() as ctx:
        wt = ctx.enter_context(nc.sbuf_tensor("wt", [K_ROWS, C], f16))
        bt = ctx.enter_context(nc.sbuf_tensor("bt", [K_ROWS, COLS], f16))
        ob = [ctx.enter_context(nc.sbuf_tensor(f"ob{i}", [128, CHUNK], f32))
              for i in range(N_CHUNKS)]
        ps = [ctx.enter_context(nc.psum_tensor(f"ps{i}", [128, CHUNK], f32))
              for i in range(2)]
        dma_w = ctx.enter_context(nc.semaphore("dma_w"))
        dma_b = ctx.enter_context(nc.semaphore("dma_b"))
        mmsem = ctx.enter_context(nc.semaphore("mmsem"))
        vcp = ctx.enter_context(nc.semaphore("vcp"))
        scp = ctx.enter_context(nc.semaphore("scp"))
        dout = ctx.enter_context(nc.semaphore("dout"))
        block = ctx.enter_context(nc.Block())

        # weights on the gpsimd ring, basis quarters on the sync ring:
        # both issue concurrently right after the preamble, and chunk k's
        # matmuls only gate on basis quarter k%4 (sync-ring FIFO order
        # makes dma_b >= 16*(q+1) imply quarters 0..q have landed)
        @block.gpsimd
        def _(gpsimd):
            gpsimd.dma_start(wt[:], mm[:]).then_inc(dma_w, 16)

        @block.sync
        def _(sync):
            for q in range(n_jo):
                sync.dma_start(bt[:, q * CHUNK:(q + 1) * CHUNK],
                               bs[:, q * CHUNK:(q + 1) * CHUNK]).then_inc(dma_b, 16)
            # chunk 0 drains at single-bank granularity (4 x [128, FREE]
            # DMAs, each issued the moment its bank is copied) so the write
            # wire starts ~1 us earlier; later chunks are full-size
            for q in range(4):
                sem, val = (vcp, q + 1) if q < 2 else (scp, q - 1)
                sync.wait_ge(sem, val)
                sync.dma_start(
                    out[0:128, q * FREE:(q + 1) * FREE],
                    ob[0][:, q * FREE:(q + 1) * FREE],
                ).then_inc(dout, 16)
            for k in range(1, N_CHUNKS):
                cblk, jo = divmod(k, n_jo)
                sync.wait_ge(vcp, k + 2)
                sync.wait_ge(scp, k + 2)
                sync.dma_start(
                    out[cblk * 128:(cblk + 1) * 128,
                        jo * CHUNK:(jo + 1) * CHUNK],
                    ob[k][:],
                ).then_inc(dout, 16)
            sync.wait_ge(dout, 16 * (N_CHUNKS + 3))

        # mmsem +1 per matmul PAIR (2 per chunk) so each half-copy starts
        # as soon as its two banks are written
        @block.tensor
        def _(tensor):
            tensor.wait_ge(dma_w, 16)
            for k in range(N_CHUNKS):
                cblk, jo = divmod(k, n_jo)
                g = k % 2
                if k < n_jo:
                    tensor.wait_ge(dma_b, 16 * (jo + 1))
                if k >= 2:
                    # psum group g freed once chunk k-2 fully copied
                    # (vcp/scp reach k exactly when chunk k-2 is drained:
                    # chunk 0 contributes 2 per engine, chunks >=1 one each)
                    tensor.wait_ge(vcp, k)
                    tensor.wait_ge(scp, k)
                for ji in range(CHUNK // FREE):
                    col = jo * CHUNK + ji * FREE
                    ins = nc.tensor.matmul(
                        ps[g][:, ji * FREE:(ji + 1) * FREE],
                        wt[:, cblk * 128:(cblk + 1) * 128],
                        bt[:, col:col + FREE],
                        start=True, stop=True,
                    )
                    # chunk 0: +1 per matmul (bank-level drains); rest: +1
                    # per pair (half-level drains)
                    if k == 0 or ji % 2 == 1:
                        ins.then_inc(mmsem, 1)

        # each chunk drains via two concurrent half-copies: DVE takes the
        # low banks, ACT the high banks
        @block.vector
        def _(vector):
            for q in range(2):          # chunk 0: banks 0,1 one at a time
                vector.wait_ge(mmsem, q + 1)
                nc.vector.tensor_copy(
                    ob[0][:, q * FREE:(q + 1) * FREE],
                    ps[0][:, q * FREE:(q + 1) * FREE]).then_inc(vcp, 1)
            for k in range(1, N_CHUNKS):
                vector.wait_ge(mmsem, 2 * k + 3)
                nc.vector.tensor_copy(
                    ob[k][:, :HALF], ps[k % 2][:, :HALF]).then_inc(vcp, 1)

        @block.scalar
        def _(scalar):
            for q in range(2, 4):       # chunk 0: banks 2,3
                scalar.wait_ge(mmsem, q + 1)
                nc.scalar.copy(
                    ob[0][:, q * FREE:(q + 1) * FREE],
                    ps[0][:, q * FREE:(q + 1) * FREE]).then_inc(scp, 1)
            for k in range(1, N_CHUNKS):
                scalar.wait_ge(mmsem, 2 * k + 4)
                nc.scalar.copy(
                    ob[k][:, HALF:], ps[k % 2][:, HALF:]).then_inc(scp, 1)

    nc.compile()
    _CACHE["nc"] = nc
    return nc


def _host_coeffs(pred_box_infra, pred_score_infra, infra_features):
    """Replicates the reference's box->(cx, cy, bev, cpf) pipeline in f32,
    then reduces to the rank-4 coefficient matrix Mmat [4, C] in f64."""
    pred_box_infra = np.asarray(pred_box_infra, dtype=np.float32)
    pred_score_infra = np.asarray(pred_score_infra, dtype=np.float32)
    feat = np.asarray(infra_features, dtype=np.float32)[0]  # [C,H,W]

    idx = np.nonzero(pred_score_infra > 0.0)[0][:MAX_N]
    if len(idx) < MAX_N:
        idx = np.concatenate([idx, np.zeros(MAX_N - len(idx), dtype=idx.dtype)])

    boxes = pred_box_infra[idx]          # [N,8,3]
    l = boxes.min(axis=1)
    r = boxes.max(axis=1)
    center = (l + r) / 2.0

    lx = (l[:, 0] + W / 2.0) / 160.0
    ly = (l[:, 1] + H / 2.0) / 160.0
    rx = (r[:, 0] + W / 2.0) / 160.0
    ry = (r[:, 1] + H / 2.0) / 160.0
    bev = (ry - ly) * (rx - lx)
    cx = (center[:, 0] + W / 2.0) / 160.0
    cy = (center[:, 1] + H / 2.0) / 160.0

    # bilinear grid-sample of feat at (cx, cy), f32 to match the reference
    ix = ((cx + 1.0) * W - 1.0) / 2.0
    iy = ((cy + 1.0) * H - 1.0) / 2.0
    x0 = np.floor(ix)
    y0 = np.floor(iy)
    x1 = x0 + 1.0
    y1 = y0 + 1.0
    wx1 = ix - x0
    wx0 = np.float32(1.0) - wx1
    wy1 = iy - y0
    wy0 = np.float32(1.0) - wy1

    def gather(xf, yf):
        xi = xf.astype(np.int32)
        yi = yf.astype(np.int32)
        valid = (xi >= 0) & (xi < W) & (yi >= 0) & (yi < H)
        xi = np.clip(xi, 0, W - 1)
        yi = np.clip(yi, 0, H - 1)
        return feat[:, yi, xi] * valid.astype(feat.dtype)

    cpf = (gather(x0, y0) * (wx0 * wy0) + gather(x1, y0) * (wx1 * wy0)
           + gather(x0, y1) * (wx0 * wy1) + gather(x1, y1) * (wx1 * wy1))

    cx64 = cx.astype(np.float64)
    cy64 = cy.astype(np.float64)
    s = 1.0 / (2.0 * bev.astype(np.float64) ** 2)
    cpf64 = cpf.astype(np.float64)
    A = cpf64 @ s / MAX_N
    B = cpf64 @ (s * (cx64 ** 2 + cy64 ** 2)) / MAX_N
    Cw = cpf64 @ (s * cx64) * (-2.0 / MAX_N)
    Ch = cpf64 @ (s * cy64) * (-2.0 / MAX_N)
    # coefficient order pairs the basis rows [r2/4 (fp16 hi), r2_lo, 1, w, h]
    coefs = np.stack([4.0 * A, A, B, Cw, Ch])          # [5, C] f64
    hi = coefs.astype(np.float16)
    lo = (coefs - hi.astype(np.float64)).astype(np.float16)
    return np.ascontiguousarray(np.concatenate([hi, lo], axis=0))  # [10, C]


def _bases():
    """Per-core fp16 basis [K_ROWS, COLS] over the core's h-slice.

    Rows (repeated for the hi and lo coefficient halves):
      q   = fp16(r2/4)  -- exactly representable (multiple of 16, <= 32640)
      r2l = r2 - 4*q    -- integer, |r2l| <= 32, exact in fp16
      1, w, h           -- integers <= 255, exact in fp16
    so the only quantization error in out = Mmat.T @ basis is the fp16
    hi/lo split of the coefficients (~2^-22 relative)."""
    if "bases" in _CACHE:
        return _CACHE["bases"]
    w = np.arange(W, dtype=np.float64)
    h = np.arange(H, dtype=np.float64)
    r2 = h[:, None] ** 2 + w[None, :] ** 2            # [H, W] integers
    q = (r2 / 4.0).astype(np.float16)
    r2l = r2 - 4.0 * q.astype(np.float64)
    basis = np.empty((5, H, W), dtype=np.float16)
    basis[0] = q
    basis[1] = r2l
    basis[2] = 1.0
    basis[3] = w[None, :]
    basis[4] = h[:, None]
    basis = np.concatenate([basis, basis], axis=0)    # [K_ROWS, H, W]
    bases = [
        np.ascontiguousarray(
            basis[:, i * H_SHARD:(i + 1) * H_SHARD, :].reshape(K_ROWS, COLS))
        for i in range(N_CORES)
    ]
    _CACHE["bases"] = bases
    return bases


def _device_run(mmat, trace=False):
    from concourse.bass_utils import run_bass_kernel_spmd

    nc = _get_nc()
    bases = _bases()
    in_maps = [{"mmat": mmat, "basis": bases[i]} for i in range(N_CORES)]
    return run_bass_kernel_spmd(nc, in_maps, core_ids=list(range(N_CORES)),
                                trace=trace)


def kernel(pred_box_infra, pred_score_infra, infra_features):
    mmat = _host_coeffs(pred_box_infra, pred_score_infra, infra_features)
    res = _device_run(mmat)
    parts = [res.results[i]["out"].reshape(C, H_SHARD, W)
             for i in range(N_CORES)]
    return np.concatenate(parts, axis=1).reshape(1, C, H, W)
